# revision 1
# baseline (speedup 1.0000x reference)
"""Trainium2 Bass kernel for nn_BatchPitNorm1d (pairwise Gaussian-CDF KDE + inverse-normal).

Math:  u[b,f] = mean_s Phi((x[b,f] - c[s,f]) / bw[f]),  out = ndtri(u),
       bw = sigmoid(bw_param).

Algorithm: for fixed f, ndtri(u) is a smooth function H_f(x) of x alone, so
instead of B*S*F pairwise Phi evals the kernel:
  1. evaluates the erf-sums g_f(t) at N=24 Chebyshev nodes, sharded
     (4 node-groups) x (2 cdf-sample-halves) over 8 cores — 6 nodes x 1024
     samples per core, one fused ACT erf instruction per node
     (accum_out = free-dim sum, per-partition scale/bias = bandwidth),
  2. AllGathers the raw sums (one 3KB/core collective), adds the halves,
  3. applies ndtri at the nodes (rational(3,3) central branch + deg-10
     log-domain tail polynomial, branchless blend),
  4. fits per-feature even/odd Chebyshev coefficients with one PE matmul,
  5. evaluates at local x via two independent all-STT Clenshaw chains
     (even/odd in w = 2*(x/XDOM)^2 - 1) on DVE.
Truncation error ~1.6e-4; total error vs the f32 reference ~5e-4 max-abs,
below the reference's own f32-vs-f64 noise (~6.8e-4).

Layout: features (F=128) on partitions; x and cdf_data arrive pre-transposed
(feature-major) from the host shard step; output returns feature-major and is
un-transposed on gather.
"""

import math
from contextlib import ExitStack

import numpy as np

import concourse.bass as bass
import concourse.bacc as bacc
import concourse.tile as tile
from concourse import mybir
from concourse import bass_utils

F32 = mybir.dt.float32

N_CORES = 8
B, S, F = 512, 2048, 128
BL = B // N_CORES          # 64 batch rows per core
N_CHEB = 24                # Chebyshev nodes / polynomial order
NGRP = 4                   # node groups (cores 2g, 2g+1 share a node group)
NSPL = 2                   # sample splits (even core: half 0, odd: half 1)
NLOC = N_CHEB // NGRP      # 8 nodes per core
SL = S // NSPL             # 1024 samples per core
XDOM = 4.6                 # Chebyshev domain [-XDOM, XDOM] must cover all x
S_CHUNK = 128              # cdf_data DMA/transpose chunk (partition dim)

# Acklam's ndtri rational approximation (rel err ~1.2e-9 in exact arithmetic).
ACK_A = [-3.969683028665376e+01, 2.209460984245205e+02, -2.759285104469687e+02,
         1.383577518672690e+02, -3.066479806614716e+01, 2.506628277459239e+00]
ACK_B = [-5.447609879822406e+01, 1.615858368580409e+02, -1.556989798598866e+02,
         6.680131188771972e+01, -1.328068155288572e+01]
ACK_C = [-7.784894002430293e-03, -3.223964580411365e-01, -2.400758277161838e+00,
         -2.549732539343734e+00, 4.374664141464968e+00, 2.938163982698783e+00]
ACK_D = [7.784695709041462e-03, 3.224671290700398e-01, 2.445134137142996e+00,
         3.754408661907416e+00]
ACK_PLOW = 0.02425

# Tail branch: ndtri(v) = P((ln v - TAIL_C)/TAIL_H) for v in [1e-10, 0.0245],
# fitted offline (deg 12, max abs err 2.9e-6 in f32 Horner).
TAIL_C = -13.367466545685957
TAIL_H = 9.6583843842545
TAIL_P = [-4.662070379682292, 1.987310755685088, 0.39160273722395583,
          0.14883020862070562, 0.06900459865562841, 0.047597883349676406,
          0.03176661817291077, -0.0067910142231145925, -0.009766979457105394,
          0.019414199950933318, 0.014492288844208983]

# Central branch: ndtri(0.5+q) = q * N(r)/D(r), r = q^2, rational (3,3)
# fitted offline (max rel err 1.0e-5).
CEN_NUM = [-14.41153095969586, 34.82754843726583, -17.684192118918105,
           2.5066372796948575]
CEN_DEN = [-14.220558591278943, 20.063397583232298, -8.101751140071201, 1.0]


def _cheb_nodes():
    th = (np.arange(N_CHEB) + 0.5) * np.pi / N_CHEB
    return (XDOM * np.cos(th)).astype(np.float32), th


def _fit_matrix():
    """Map H-at-nodes -> even/odd coefficients.

    Basis: T_j(w) (j<N/2) and xt*T_j(w) (j<N/2), w = 2*xt^2-1, xt = x/XDOM.
    Returns Cfit[n, k] with columns 0..N/2-1 = beta (even), N/2..N-1 = gamma
    (odd), so alpha = H_nodes^T @ Cfit via the PE matmul.
    """
    _, th = _cheb_nodes()
    xt = np.cos(th)                      # normalized nodes
    w = 2 * xt * xt - 1
    J = N_CHEB // 2
    M = np.zeros((N_CHEB, N_CHEB))
    for j in range(J):
        M[:, j] = np.cos(j * np.arccos(np.clip(w, -1, 1)))
        M[:, J + j] = xt * M[:, j]
    Minv = np.linalg.inv(M)              # coeffs = Minv @ H
    return np.ascontiguousarray(Minv.T).astype(np.float32)


def _tt(nc, pool, in0, in1, op, name, tag=None):
    """Two-tensor op emitted as scalar_tensor_tensor (in0+0) op in1 —
    InstTensorScalarPtr supports the DVE 2x SBUF perf mode, InstTensorTensor
    does not."""
    t = pool.tile([in0.shape[0], in0.shape[1]], F32, name=name, tag=tag or name)
    nc.vector.scalar_tensor_tensor(out=t, in0=in0, scalar=0.0, in1=in1,
                                   op0=mybir.AluOpType.add, op1=op)
    return t


def _horner(nc, pool, r, coeffs, name):
    """Evaluate sum_j coeffs[j] * r^(J-1-j) via STT-fused Horner.

    acc_{j+1} = (acc_j + coeffs[j]) * r   [one scalar_tensor_tensor each],
    then a final tensor_scalar add of coeffs[-1].
    """
    p, w = r.shape[0], r.shape[1]
    acc = pool.tile([p, w], F32, name=f"{name}_h0", tag=f"{name}_h")
    nc.vector.tensor_scalar(out=acc, in0=r, scalar1=float(coeffs[0]), scalar2=None,
                            op0=mybir.AluOpType.mult)
    for j, cj in enumerate(coeffs[1:-1]):
        acc2 = pool.tile([p, w], F32, name=f"{name}_h{j + 1}", tag=f"{name}_h")
        nc.vector.scalar_tensor_tensor(out=acc2, in0=acc, scalar=float(cj),
                                       in1=r, op0=mybir.AluOpType.add,
                                       op1=mybir.AluOpType.mult)
        acc = acc2
    accf = pool.tile([p, w], F32, name=f"{name}_hf", tag=f"{name}_h")
    nc.vector.tensor_scalar(out=accf, in0=acc, scalar1=float(coeffs[-1]),
                            scalar2=None, op0=mybir.AluOpType.add)
    return accf


def _emit_ndtri(nc, pool, g, width, P=128, gscale=1.0):
    """Branchless ndtri(0.5 + g*gscale) on a [P, width] tile of raw erf-sums.

    Central branch: q*N(r)/D(r) rational(3,3); tail: deg-10 polynomial in
    (ln v - C)/H (needs only the Ln table set, no exp/sqrt).  The two DVE
    chains are independent and emitted interleaved; Ln runs on ACT.
    """
    ADD, MUL, SUB = (mybir.AluOpType.add, mybir.AluOpType.mult,
                     mybir.AluOpType.subtract)

    def ts(eng, name, in0, s1, s2=None, op0=MUL, op1=ADD):
        t = pool.tile([P, width], F32, name=name, tag=name)
        if s2 is None:
            eng.tensor_scalar(out=t, in0=in0, scalar1=s1, scalar2=None, op0=op0)
        else:
            eng.tensor_scalar(out=t, in0=in0, scalar1=s1, scalar2=s2,
                              op0=op0, op1=op1)
        return t

    def stt(eng, name, in0, s, in1, op0, op1):
        t = pool.tile([P, width], F32, name=name, tag=name)
        eng.scalar_tensor_tensor(out=t, in0=in0, scalar=s, in1=in1,
                                 op0=op0, op1=op1)
        return t

    def horner(eng, key, xvar, coeffs):
        acc = ts(eng, f"{key}0", xvar, float(coeffs[0]))
        for i, cj in enumerate(coeffs[1:-1]):
            acc = stt(eng, f"{key}{i + 1}", acc, float(cj), xvar, ADD, MUL)
        return ts(eng, f"{key}f", acc, 1.0, float(coeffs[-1]))

    dve, gps = nc.vector, nc.vector

    # prolog (DVE): all four derived ops read g directly (no serial chain)
    u = ts(dve, "ung", g, gscale, 0.5)
    omu = ts(dve, "omu", g, -gscale, 0.5)
    v = stt(dve, "v", u, 1e-10, omu, mybir.AluOpType.max, mybir.AluOpType.min)
    lnv = pool.tile([P, width], F32, name="lnv")
    nc.scalar.activation(out=lnv, in_=v, func=mybir.ActivationFunctionType.Ln)
    mge = ts(dve, "mge", g, 0.0, None, op0=mybir.AluOpType.is_ge)
    nsgn = ts(dve, "nsgn", mge, -2.0, 1.0)

    # two independent DVE chains, emitted interleaved so the per-instruction
    # write-ack pipelines across them
    st = {}
    tai = []
    tai.append(lambda: st.update(m=ts(dve, "mt", lnv, 1.0 / TAIL_H,
                                      -TAIL_C / TAIL_H)))
    tai.append(lambda: st.update(tp=ts(dve, "tp0", st["m"],
                                       float(TAIL_P[::-1][0]))))
    for _i, _cj in enumerate(TAIL_P[::-1][1:-1]):
        tai.append(lambda _cj=_cj, _i=_i: st.update(
            tp=stt(dve, f"tp{_i + 1}", st["tp"], float(_cj), st["m"], ADD, MUL)))
    tai.append(lambda: st.update(tp=ts(dve, "tpf", st["tp"], 1.0,
                                       float(TAIL_P[0]), op0=MUL, op1=ADD)))
    tai.append(lambda: st.update(xt=stt(dve, "xtl", st["tp"], 0.0, nsgn,
                                        ADD, MUL)))

    cen = []
    cen.append(lambda: st.update(q=ts(dve, "qc", g, gscale, None, op0=MUL)))
    cen.append(lambda: st.update(r=stt(dve, "rc", st["q"], 0.0, st["q"],
                                       ADD, MUL)))
    cen.append(lambda: st.update(cn=ts(dve, "cn0", st["r"], float(CEN_NUM[0]))))
    for _i, _cj in enumerate(CEN_NUM[1:-1]):
        cen.append(lambda _cj=_cj, _i=_i: st.update(
            cn=stt(dve, f"cn{_i + 1}", st["cn"], float(_cj), st["r"], ADD, MUL)))
    cen.append(lambda: st.update(cn=ts(dve, "cnf", st["cn"], 1.0,
                                       float(CEN_NUM[-1]), op0=MUL, op1=ADD)))
    cen.append(lambda: st.update(cd=ts(dve, "cd0", st["r"], float(CEN_DEN[0]))))
    for _i, _cj in enumerate(CEN_DEN[1:-1]):
        cen.append(lambda _cj=_cj, _i=_i: st.update(
            cd=stt(dve, f"cd{_i + 1}", st["cd"], float(_cj), st["r"], ADD, MUL)))
    cen.append(lambda: st.update(cd=ts(dve, "cdf_", st["cd"], 1.0,
                                       float(CEN_DEN[-1]), op0=MUL, op1=ADD)))

    def _recip():
        t = pool.tile([P, width], F32, name="cdi")
        dve.reciprocal(out=t, in_=st["cd"])
        st["cdi"] = t
    cen.append(_recip)
    cen.append(lambda: st.update(nq=stt(dve, "nq", st["cn"], 0.0, st["q"],
                                        ADD, MUL)))
    cen.append(lambda: st.update(xc=stt(dve, "xc", st["nq"], 0.0, st["cdi"],
                                        ADD, MUL)))

    while cen or tai:
        if cen:
            cen.pop(0)()
        if tai:
            tai.pop(0)()
    xt, xc = st["xt"], st["xc"]

    # blend: h = xt + [v >= PLOW]*(xc - xt)   (DVE)
    mc = ts(dve, "mcsel", v, float(ACK_PLOW), None, op0=mybir.AluOpType.is_ge)
    d = stt(dve, "dsel", xc, 0.0, xt, ADD, SUB)
    md = stt(dve, "mdsel", mc, 0.0, d, ADD, MUL)
    h = stt(dve, "hout", xt, 0.0, md, ADD, ADD)
    return h


def build(with_collective=True, stages=("load", "grid", "ndtri", "gather", "fit",
                                        "clenshaw", "store"), debug_taps=False,
          repeat=1):
    stages = set(stages)
    nc = bacc.Bacc("TRN2", target_bir_lowering=False, debug=False,
                   enable_asserts=False, num_devices=N_CORES)

    # Inputs arrive pre-transposed (feature-major) from the host shard step.
    x_t = nc.dram_tensor("x_t", [F, BL], F32, kind="ExternalInput")
    cdf_t = nc.dram_tensor("cdf_t", [F, SL], F32, kind="ExternalInput")
    bwp = nc.dram_tensor("bwp", [1, F], F32, kind="ExternalInput")
    tloc = nc.dram_tensor("tloc", [1, NLOC], F32, kind="ExternalInput")
    out = nc.dram_tensor("out", [F, BL], F32, kind="ExternalOutput")
    taps = {}
    if debug_taps:
        for nm, shp in [("d_gacc", [F, NLOC]), ("d_gsum", [N_CHEB, F]),
                        ("d_u", [N_CHEB, F]), ("d_h", [N_CHEB, F]),
                        ("d_alpha", [F, N_CHEB]), ("d_xt1", [F, BL])]:
            taps[nm] = nc.dram_tensor(nm, shp, F32, kind="ExternalOutput")

    cfit_h = nc.inline_tensor(_fit_matrix(), name="cfit")

    with tile.TileContext(nc) as tc, ExitStack() as ctx:
        io = ctx.enter_context(tc.tile_pool(name="io", bufs=2))
        small = ctx.enter_context(tc.tile_pool(name="small", bufs=1))
        nd = ctx.enter_context(tc.tile_pool(name="nd", bufs=3))
        psum = ctx.enter_context(tc.tile_pool(name="psum", bufs=2, space="PSUM"))
        dram = ctx.enter_context(tc.tile_pool(name="dram", bufs=1, space="DRAM"))
        clen = ctx.enter_context(tc.tile_pool(name="clen", bufs=6))

        # --- constants / small inputs
        cfit_sb = small.tile([N_CHEB, N_CHEB], F32)
        nc.scalar.dma_start(out=cfit_sb, in_=cfit_h[:, :])
        bw_col = small.tile([F, 1], F32)
        nc.scalar.dma_start(out=bw_col, in_=bwp.ap().rearrange("o f -> f o"))
        t_bc = small.tile([F, NLOC], F32)
        nc.scalar.dma_start(
            out=t_bc,
            in_=bass.AP(tensor=tloc, offset=0, ap=[[0, F], [1, NLOC]]),
        )

        # --- bandwidth scalars: a = 1/(sigmoid(bwp)*sqrt(2)); neg_a = -a
        bw_sig = small.tile([F, 1], F32)
        nc.scalar.activation(out=bw_sig, in_=bw_col,
                             func=mybir.ActivationFunctionType.Sigmoid)
        inv_bw = small.tile([F, 1], F32)
        nc.vector.reciprocal(out=inv_bw, in_=bw_sig)
        a_col = small.tile([F, 1], F32)
        nc.vector.tensor_scalar(out=a_col, in0=inv_bw, scalar1=1.0 / math.sqrt(2.0),
                                scalar2=None, op0=mybir.AluOpType.mult)
        neg_a = small.tile([F, 1], F32)
        nc.vector.tensor_scalar(out=neg_a, in0=a_col, scalar1=-1.0,
                                scalar2=None, op0=mybir.AluOpType.mult)
        # bias_all[f, j] = a_f * t_j
        bias_all = small.tile([F, NLOC], F32)
        nc.vector.tensor_scalar_mul(out=bias_all, in0=t_bc, scalar1=a_col)

        # --- bulk loads (already feature-major; no transposes needed)
        cT = io.tile([F, SL], F32)
        if "load" in stages:
            nc.sync.dma_start(out=cT, in_=cdf_t[:, :])
        else:
            nc.vector.memset(cT, 0.0)
        x_sb = io.tile([F, BL], F32)
        nc.gpsimd.dma_start(out=x_sb, in_=x_t[:, :])
        xt0 = small.tile([F, BL], F32)
        nc.vector.tensor_scalar(out=xt0, in0=x_sb, scalar1=1.0 / XDOM, scalar2=None,
                                op0=mybir.AluOpType.mult)
        xt1 = small.tile([F, BL], F32)  # clamp to [-1, 1]: off-domain x degrades
        nc.vector.tensor_scalar(out=xt1, in0=xt0, scalar1=1.0, scalar2=-1.0,
                                op0=mybir.AluOpType.min, op1=mybir.AluOpType.max)

        for _rep in range(repeat):
            # --- grid pass: gacc[f, j] = sum_s erf(a_f * (t_j - c_sf)) over the
            # local sample half (ACT, the only O(S) work)
            gacc = nd.tile([F, NLOC], F32, tag="gacc")
            scratch = psum.tile([128, SL], F32, tag="scr", bufs=1)
            if "grid" not in stages:
                nc.vector.memset(gacc, 0.0)
            for j in range(NLOC if "grid" in stages else 0):
                nc.scalar.activation(out=scratch, in_=cT,
                                     func=mybir.ActivationFunctionType.Erf,
                                     bias=bias_all[:, j:j + 1], scale=neg_a,
                                     accum_out=gacc[:, j:j + 1])

            # --- exchange: write gacc^T as [NLOC, F], AllGather (block order =
            # replica rank 2g + h), sum the two sample-halves -> g_sum [N, F]
            cin = dram.tile([NLOC, F], F32, tag=f"cin{_rep}")
            g_sum = nd.tile([N_CHEB, F], F32)
            if "gather" in stages:
                nc.sync.dma_start(out=cin.rearrange("n f -> f n"), in_=gacc)
                cout = dram.tile([N_CORES, NLOC, F], F32,
                                 addr_space="Shared" if with_collective else "Local")
                if with_collective:
                    nc.gpsimd.collective_compute(
                        "AllGather", mybir.AluOpType.bypass,
                        replica_groups=[list(range(N_CORES))],
                        ins=[cin.opt()], outs=[cout.opt()],
                    )
                gh = [nd.tile([N_CHEB, F], F32, name=f"gh{h}", tag=f"gh{h}")
                      for h in range(NSPL)]
                # readback: partition n = g*NLOC + row, skipping over the other half
                for h in range(NSPL):
                    if with_collective:
                        src_ap = bass.AP(
                            tensor=cout.tensor, offset=cout.offset + h * NLOC * F,
                            ap=[[NSPL * NLOC * F, NGRP], [F, NLOC], [1, F]])
                    else:  # stand-in: broadcast-read own block (timing model only)
                        src_ap = bass.AP(tensor=cin.tensor, offset=cin.offset,
                                         ap=[[0, NGRP], [F, NLOC], [1, F]])
                    (nc.scalar if h == 0 else nc.sync).dma_start(
                        out=gh[h][:, :], in_=src_ap)
                nc.vector.scalar_tensor_tensor(
                    out=g_sum, in0=gh[0], scalar=0.0, in1=gh[1],
                    op0=mybir.AluOpType.add, op1=mybir.AluOpType.add)
            else:
                nc.vector.memset(g_sum, 0.0)

            # H = ndtri(0.5 + g/(2S)) on [N, F] (every core, redundantly)
            if "ndtri" in stages:
                h_nodes = _emit_ndtri(nc, nd, g_sum, F, P=N_CHEB,
                                      gscale=1.0 / (2.0 * S))
            else:
                h_nodes = nd.tile([N_CHEB, F], F32, name="u_nodes")
                nc.vector.tensor_scalar(out=h_nodes, in0=g_sum,
                                        scalar1=1.0 / (2.0 * S), scalar2=0.5,
                                        op0=mybir.AluOpType.mult,
                                        op1=mybir.AluOpType.add)

            # --- fit: alpha[f, k] = sum_n H[n, f] * Cfit[n, k]  (one matmul)
            alpha = nd.tile([F, N_CHEB], F32, tag="alpha")
            if "fit" in stages:
                alpha_ps = psum.tile([F, N_CHEB], F32, tag="mm")
                nc.tensor.matmul(out=alpha_ps, lhsT=h_nodes, rhs=cfit_sb,
                                 start=True, stop=True)
                nc.vector.tensor_copy(out=alpha, in_=alpha_ps)
            else:
                nc.vector.memset(alpha, 0.0)

            # --- Clenshaw, even/odd split: y = pe(w) + xt*po(w), w = 2*xt^2-1.
            # Two independent all-STT chains pipeline on DVE without stalling on
            # the per-instruction write-ack.
            xsq = _tt(nc, clen, xt1, xt1, mybir.AluOpType.mult, "xsq")
            wt = clen.tile([F, BL], F32)
            nc.vector.tensor_scalar(out=wt, in0=xsq, scalar1=2.0, scalar2=-1.0,
                                    op0=mybir.AluOpType.mult, op1=mybir.AluOpType.add)
            wt2 = clen.tile([F, BL], F32)
            nc.vector.tensor_scalar(out=wt2, in0=wt, scalar1=2.0, scalar2=None,
                                    op0=mybir.AluOpType.mult)
            J = N_CHEB // 2
            nsteps = J - 1 if "clenshaw" in stages else 0

            def chain(name, col):
                # peeled steps j = J-1 (b=alpha bcast) and J-2 (no subtract)
                b1 = clen.tile([F, BL], F32, name=f"{name}_b0", tag=f"{name}_b")
                nc.vector.tensor_scalar(out=b1, in0=xt1, scalar1=0.0,
                                        scalar2=alpha[:, col + nsteps:col + nsteps + 1],
                                        op0=mybir.AluOpType.mult,
                                        op1=mybir.AluOpType.add)
                b2 = b1
                bn = clen.tile([F, BL], F32, name=f"{name}_c0", tag=f"{name}_c")
                nc.vector.scalar_tensor_tensor(
                    out=bn, in0=wt2, scalar=0.0, in1=b1,
                    op0=mybir.AluOpType.add, op1=mybir.AluOpType.mult)
                bn2 = clen.tile([F, BL], F32, name=f"{name}_b1", tag=f"{name}_b")
                nc.vector.tensor_scalar(out=bn2, in0=bn, scalar1=1.0,
                                        scalar2=alpha[:, col + nsteps - 1:col + nsteps],
                                        op0=mybir.AluOpType.mult,
                                        op1=mybir.AluOpType.add)
                return [bn2, b1]

            ce = chain("ce", 0); co = chain("co", J)
            for j in range(nsteps - 2, 0, -1):
                for name, ch, col in (("ce", ce, j), ("co", co, J + j)):
                    b1, b2 = ch
                    p = clen.tile([F, BL], F32, name=f"{name}_p{j}", tag=f"{name}_p")
                    nc.vector.scalar_tensor_tensor(out=p, in0=b1, scalar=0.0, in1=wt2,
                                                   op0=mybir.AluOpType.add,
                                                   op1=mybir.AluOpType.mult)
                    bn = clen.tile([F, BL], F32, name=f"{name}_b{j}", tag=f"{name}_b")
                    nc.vector.scalar_tensor_tensor(out=bn, in0=p,
                                                   scalar=alpha[:, col:col + 1],
                                                   in1=b2, op0=mybir.AluOpType.add,
                                                   op1=mybir.AluOpType.subtract)
                    ch[1] = b1; ch[0] = bn
            # final step with w (not 2w)
            res = []
            for name, ch, col in (("ce", ce, 0), ("co", co, J)):
                b1, b2 = ch
                p = clen.tile([F, BL], F32, name=f"{name}_pf", tag=f"{name}_p")
                nc.vector.scalar_tensor_tensor(out=p, in0=b1, scalar=0.0, in1=wt,
                                               op0=mybir.AluOpType.add,
                                               op1=mybir.AluOpType.mult)
                r = clen.tile([F, BL], F32, name=f"{name}_r", tag=f"{name}_b")
                nc.vector.scalar_tensor_tensor(out=r, in0=p,
                                               scalar=alpha[:, col:col + 1],
                                               in1=b2, op0=mybir.AluOpType.add,
                                               op1=mybir.AluOpType.subtract)
                res.append(r)
            ye, yo = res
            xyo = _tt(nc, clen, yo, xt1, mybir.AluOpType.mult, "xyo")
            y = _tt(nc, clen, ye, xyo, mybir.AluOpType.add, "yfin")

            # --- store feature-major; the host un-transposes during gather
            nc.sync.dma_start(out=out[:, :], in_=y)

        if debug_taps:
            for h in range(NSPL):
                dt_ = nc.dram_tensor(f"d_gh{h}", [N_CHEB, F], F32,
                                     kind="ExternalOutput")
                nc.sync.dma_start(out=dt_[:, :], in_=gh[h])
            for nm, t in [("d_gacc", gacc), ("d_gsum", g_sum), ("d_u", u_nodes),
                          ("d_h", h_nodes), ("d_alpha", alpha), ("d_xt1", xt1)]:
                nc.sync.dma_start(out=taps[nm][:, :], in_=t)

    nc.compile()
    return nc


_CACHE = {}


def _get_nc():
    if "nc" not in _CACHE:
        _CACHE["nc"] = build(with_collective=True)
    return _CACHE["nc"]


def kernel(x, cdf_data, bw_param):
    x = np.ascontiguousarray(x, dtype=np.float32)
    cdf_data = np.ascontiguousarray(cdf_data, dtype=np.float32)
    bw_param = np.ascontiguousarray(bw_param, dtype=np.float32)
    nc = _get_nc()
    nodes, _ = _cheb_nodes()
    xt = np.ascontiguousarray(x.T)                      # [F, B]
    cdf_halves = [np.ascontiguousarray(cdf_data[h * SL:(h + 1) * SL].T)
                  for h in range(NSPL)]                  # each [F, SL]
    in_maps = []
    for i in range(N_CORES):
        g, h = i // NSPL, i % NSPL
        in_maps.append({
            "x_t": np.ascontiguousarray(xt[:, i * BL:(i + 1) * BL]),
            "cdf_t": cdf_halves[h],
            "bwp": bw_param,
            "tloc": nodes[g * NLOC:(g + 1) * NLOC].reshape(1, NLOC),
        })
    res = bass_utils.run_bass_kernel_spmd(nc, in_maps, core_ids=list(range(N_CORES)))
    return np.concatenate([res.results[i]["out"].T for i in range(N_CORES)], axis=0)



# revision 7
# speedup vs baseline: 1.3905x; 1.3905x over previous
"""Trainium2 Bass kernel for nn_BatchPitNorm1d (pairwise Gaussian-CDF KDE + inverse-normal).

Math:  u[b,f] = mean_s Phi((x[b,f] - c[s,f]) / bw[f]),  out = ndtri(u),
       bw = sigmoid(bw_param).

Algorithm (v2): for fixed f, ndtri(u) is a smooth function H_f(x) of x alone,
so instead of B*S*F pairwise Phi evals the kernel:
  1. evaluates the erf-sums g_f(t) at N=12 Chebyshev nodes on a runtime-tight
     domain [-XD, XD] (XD = max|x|), sharded (4 node-groups) x (2 sample
     halves) over 8 cores - NLOC=3 nodes x 1024 samples per core, one fused
     ACT erf instruction per node (accum_out = free-dim sum, per-partition
     scale/bias precomputed on host),
  2. AllGathers the raw [F, NLOC] blocks (feature-major), reads them back
     with one 4D-AP DMA as [F, 2N] and adds the two sample-halves,
  3. applies ndtri at the nodes in feature-major [F, N] layout: central
     rational(3,3) on DVE, deg-5 log-domain tail polynomial on GpSimd (Pool),
     Ln on ACT (table load hidden under the gather), branchless blend via
     copy_predicated,
  4. transposes H via PE, fits per-feature even/odd Chebyshev coefficients
     with one PE matmul,
  5. evaluates y = sum_k ae_k T_k(w) + x~ * sum_k ao_k T_k(w), w = 2x~^2-1,
     with basis tiles T_k / x~T_k precomputed during the grid phase and two
     parallel per-partition-scalar accumulation chains (DVE even, Pool odd).

Host-side prep (cheap [F]-sized math): transpose/shard, x~ = x/XD, w, bw ->
erf scale/bias vectors.  Total error vs the f32 reference: rel ~9e-4
(gate 2e-2).
"""

import math
from contextlib import ExitStack

import numpy as np

import concourse.bass as bass
import concourse.bacc as bacc
import concourse.tile as tile
from concourse import mybir
from concourse import bass_utils

F32 = mybir.dt.float32
ADD = mybir.AluOpType.add
MUL = mybir.AluOpType.mult
SUB = mybir.AluOpType.subtract

N_CORES = 8
B, S, F = 512, 2048, 128
BL = B // N_CORES          # 64 batch rows per core
N_CHEB = 12                # Chebyshev nodes / polynomial order
NGRP = 4                   # node groups (cores 2g, 2g+1 share a node group)
NSPL = 2                   # sample splits (even core: half 0, odd: half 1)
NLOC = N_CHEB // NGRP      # 3 nodes per core
SL = S // NSPL             # 1024 samples per core
J = N_CHEB // 2            # even/odd coefficient count

GSCALE = 1.0 / (2.0 * S)
PLOW = 0.02425             # central/tail blend point (on v = min(u,1-u))
VCLAMP = 0.5 - 2.5e-6      # |q| clamp => v >= 2.5e-6 (empirical node min 5e-6)

# Central branch: ndtri(0.5+q) = q*N(r)/D(r), r = q^2, rational (3,3)
# fitted offline for v >= PLOW (max rel err ~1e-5).
CEN_NUM = [-14.41153095969586, 34.82754843726583, -17.684192118918105,
           2.5066372796948575]
CEN_DEN = [-14.220558591278943, 20.063397583232298, -8.101751140071201, 1.0]

# Tail branch: ndtri(v) = P(m), m = (ln v - TAIL_C)/TAIL_H, fitted on
# v in [1.5e-6, 0.0295] (deg 5, max abs err 3.8e-4).  Coeffs low->high.
TAIL_C = -8.466705232746236
TAIL_H = 4.943340217109873
TAIL_PW = [-3.5264766814047905, 1.3094044377238931, 0.20960029243044923,
           0.06417554320766394, 0.037068735313100606, 0.017653539365858555]


def _cheb_theta():
    return (np.arange(N_CHEB) + 0.5) * np.pi / N_CHEB


def _fit_matrix():
    """Cfit[n, k] with alpha[f, k] = sum_n H[f, n] * Cfit[n, k].

    Basis columns 0..J-1 = even coeffs (T_j(w)), J..N-1 = odd (xt*T_j(w)),
    w = 2*xt^2-1, xt = normalized nodes.  XD-independent.
    """
    th = _cheb_theta()
    xt = np.cos(th)
    w = 2 * xt * xt - 1
    M = np.zeros((N_CHEB, N_CHEB))
    for j in range(J):
        M[:, j] = np.cos(j * np.arccos(np.clip(w, -1, 1)))
        M[:, J + j] = xt * M[:, j]
    return np.ascontiguousarray(np.linalg.inv(M).T).astype(np.float32)


def build(with_collective=True, debug_taps=False):
    nc = bacc.Bacc("TRN2", target_bir_lowering=False, debug=False,
                   enable_asserts=False, num_devices=N_CORES)

    # Inputs arrive pre-transposed (feature-major) from the host shard step.
    xw = nc.dram_tensor("xw", [F, 2 * BL], F32, kind="ExternalInput")       # xt | wt
    cdf_t = nc.dram_tensor("cdf_t", [F, SL], F32, kind="ExternalInput")
    consts = nc.dram_tensor("consts", [F, 1 + NLOC], F32, kind="ExternalInput")  # -a | a*t_j
    out = nc.dram_tensor("out", [F, BL], F32, kind="ExternalOutput")
    taps = {}
    if debug_taps:
        for nm, shp in [("d_gacc", [F, NLOC]), ("d_gsum", [F, N_CHEB]),
                        ("d_h", [F, N_CHEB]), ("d_alpha", [F, N_CHEB]),
                        ("d_acce", [F, BL]), ("d_acco", [F, BL])]:
            taps[nm] = nc.dram_tensor(nm, shp, F32, kind="ExternalOutput")

    cfit_h = nc.inline_tensor(_fit_matrix(), name="cfit")
    ident_h = nc.inline_tensor(np.eye(F, dtype=np.float32), name="ident")

    with tile.TileContext(nc) as tc, ExitStack() as ctx:
        sb = ctx.enter_context(tc.tile_pool(name="sb", bufs=1))
        psum = ctx.enter_context(tc.tile_pool(name="psum", bufs=1, space="PSUM"))
        dram = ctx.enter_context(tc.tile_pool(name="dram", bufs=1, space="DRAM"))

        D = nc.vector    # DVE
        P = nc.gpsimd    # Pool
        A = nc.scalar    # ACT
        SP = nc.sync     # SP

        def ts(eng, name, in0, s1, s2=None, op0=MUL, op1=ADD, w=N_CHEB):
            t = sb.tile([F, w], F32, name=name, tag=name)
            if s2 is None:
                eng.tensor_scalar(out=t, in0=in0, scalar1=s1, scalar2=None, op0=op0)
            else:
                eng.tensor_scalar(out=t, in0=in0, scalar1=s1, scalar2=s2,
                                  op0=op0, op1=op1)
            return t

        def stt(eng, name, in0, s, in1, op0=ADD, op1=MUL, w=N_CHEB, out=None):
            t = out if out is not None else sb.tile([F, w], F32, name=name, tag=name)
            eng.scalar_tensor_tensor(out=t, in0=in0, scalar=s, in1=in1,
                                     op0=op0, op1=op1)
            return t

        # ---------------- input DMAs (one per queue, issued up front)
        cT = sb.tile([F, SL], F32, name="cT")
        SP.dma_start(out=cT[:, :SL // 2], in_=cdf_t[:, :SL // 2])
        A.dma_start(out=cT[:, SL // 2:], in_=cdf_t[:, SL // 2:])
        xw_sb = sb.tile([F, 2 * BL], F32, name="xw")
        P.dma_start(out=xw_sb, in_=xw[:, :])
        cst = sb.tile([F, 1 + NLOC], F32, name="cst")
        P.dma_start(out=cst, in_=consts[:, :])
        cfit_sb = sb.tile([N_CHEB, N_CHEB], F32, name="cfit")
        P.dma_start(out=cfit_sb, in_=cfit_h[:, :])
        ident_sb = sb.tile([F, F], F32, name="ident")
        P.dma_start(out=ident_sb, in_=ident_h[:, :])

        xt = xw_sb[:, :BL]
        wt = xw_sb[:, BL:]

        half_c = sb.tile([F, 1], F32, name="halfc")
        D.memset(half_c, 0.5)

        # ---------------- basis precompute (hidden under grid phase)
        # Even basis T_k(w), odd basis xt*T_k(w); T0=1 and xT0=xt are implicit.
        wt2 = ts(D, "wt2", wt, 2.0, w=BL)
        wsq = stt(D, "wsq", wt, 0.0, wt, w=BL)
        T2 = ts(D, "T2", wsq, 2.0, -1.0, w=BL)
        Tk = {1: wt, 2: T2}
        for k in range(3, J):
            p = stt(D, f"Tp{k}", Tk[k - 1], 0.0, wt2, w=BL)
            Tk[k] = stt(D, f"T{k}", p, 0.0, Tk[k - 2], op1=SUB, w=BL)
        xTk = {}
        for k in range(1, J):
            xTk[k] = stt(P, f"xT{k}", Tk[k], 0.0, xt, w=BL)

        # ---------------- grid: gacc[f, j] = sum_s erf(-a_f*c_sf + a_f*t_j)
        gacc = sb.tile([F, NLOC], F32, name="gacc")
        scr = psum.tile([F, SL], F32, name="scr", tag="scr")
        for j in range(NLOC):
            A.activation(out=scr, in_=cT, func=mybir.ActivationFunctionType.Erf,
                         bias=cst[:, 1 + j:2 + j], scale=cst[:, 0:1],
                         accum_out=gacc[:, j:j + 1])
        # Force the Ln table switch now so the ~1.3us load hides under the
        # gather round-trip (a data-independent dummy Ln on ACT).
        lndum = sb.tile([F, 1], F32, name="lndum")
        A.activation(out=lndum, in_=cst[:, 0:1],
                     func=mybir.ActivationFunctionType.Ln, scale=0.0,
                     bias=half_c[:, 0:1])

        # ---------------- exchange: AllGather of the [F, NLOC] blocks
        cin = dram.tile([F, NLOC], F32, tag="cin")
        SP.dma_start(out=cin[:, :], in_=gacc)
        cout = dram.tile([N_CORES, F, NLOC], F32, tag="cout",
                         addr_space="Shared" if with_collective else "Local")
        if with_collective:
            P.collective_compute(
                "AllGather", mybir.AluOpType.bypass,
                replica_groups=[list(range(N_CORES))],
                ins=[cin.opt()], outs=[cout.opt()],
            )
        gbig = sb.tile([F, NSPL * N_CHEB], F32, name="gbig")
        for h in range(NSPL):
            if with_collective:
                src_ap = bass.AP(
                    tensor=cout.tensor, offset=cout.offset + h * F * NLOC,
                    ap=[[NLOC, F], [NSPL * F * NLOC, NGRP], [1, NLOC]])
            else:  # stand-in: broadcast-read own block (timing model only)
                src_ap = bass.AP(
                    tensor=cin.tensor, offset=cin.offset,
                    ap=[[NLOC, F], [0, NGRP], [1, NLOC]])
            (A if h == 0 else SP).dma_start(
                out=gbig[:, h * N_CHEB:(h + 1) * N_CHEB], in_=src_ap)

        # g_sum[f, n] = sum over the NSPL halves
        g_sum = stt(D, "gsum", gbig[:, :N_CHEB], 0.0, gbig[:, N_CHEB:], op1=ADD)

        # ---------------- ndtri at the nodes, feature-major [F, N]
        # DVE: q, r, |q|-clamp, central mask + rational(3,3)
        q = ts(D, "q", g_sum, GSCALE)
        r = stt(D, "r", q, 0.0, q)
        mn2 = ts(D, "mn2", q, 0.0, VCLAMP, op0=mybir.AluOpType.abs_max,
                 op1=mybir.AluOpType.min)
        mc = ts(D, "mc", mn2, 0.5 - PLOW, None, op0=mybir.AluOpType.is_le)
        # ACT: lnv = Ln(0.5 - |q|)
        lnv = sb.tile([F, N_CHEB], F32, name="lnv")
        A.activation(out=lnv, in_=mn2, func=mybir.ActivationFunctionType.Ln,
                     scale=-1.0, bias=half_c[:, 0:1])
        # central: q*N(r)/D(r)
        ca = ts(D, "ca0", r, float(CEN_NUM[0]))
        ca = stt(D, "ca1", ca, float(CEN_NUM[1]), r)
        ca = stt(D, "ca2", ca, float(CEN_NUM[2]), r)
        nq = stt(D, "nq", ca, float(CEN_NUM[3]), q)
        da = ts(D, "da0", r, float(CEN_DEN[0]))
        da = stt(D, "da1", da, float(CEN_DEN[1]), r)
        da = stt(D, "da2", da, float(CEN_DEN[2]), r)
        df = ts(D, "df", da, float(CEN_DEN[3]), None, op0=ADD)
        rec = sb.tile([F, N_CHEB], F32, name="rec")
        D.reciprocal(out=rec, in_=df)
        xc = stt(D, "xc", nq, 0.0, rec)
        # tail on Pool: P(m)*nsgn, m = (lnv - C)/H
        mge = ts(P, "mge", g_sum, 0.0, None, op0=mybir.AluOpType.is_ge)
        nsgn = ts(P, "nsgn", mge, -2.0, 1.0)
        m = ts(P, "m", lnv, 1.0 / TAIL_H, -TAIL_C / TAIL_H)
        hc = TAIL_PW[::-1]  # high -> low
        ta = ts(P, "ta0", m, float(hc[0]))
        for i, c in enumerate(hc[1:-1]):
            ta = stt(P, f"ta{i + 1}", ta, float(c), m)
        h = sb.tile([F, N_CHEB], F32, name="h")
        stt(P, "tsgn", ta, float(hc[-1]), nsgn, out=h)
        # blend: overwrite central region with xc
        D.copy_predicated(h, mc, xc)

        # ---------------- fit: alpha = h @ Cfit via PE transpose + matmul
        hT_ps = psum.tile([N_CHEB, F], F32, tag="hT")
        nc.tensor.transpose(hT_ps, h, ident_sb)
        hT_sb = sb.tile([N_CHEB, F], F32, name="hT")
        P.tensor_copy(out=hT_sb, in_=hT_ps)
        alpha_ps = psum.tile([F, N_CHEB], F32, tag="alpha")
        nc.tensor.matmul(out=alpha_ps, lhsT=hT_sb, rhs=cfit_sb,
                         start=True, stop=True)
        alpha = sb.tile([F, N_CHEB], F32, name="alpha")
        D.tensor_copy(out=alpha, in_=alpha_ps)

        # ---------------- evaluate: two parallel accumulation chains
        acc_e = ts(D, "ae1", wt, alpha[:, 1:2], None, w=BL)
        for k in range(2, J):
            acc_e = stt(D, f"ae{k}", Tk[k], alpha[:, k:k + 1], acc_e,
                        op0=MUL, op1=ADD, w=BL)
        acc_o = ts(P, "ao0", xt, alpha[:, J:J + 1], None, w=BL)
        for k in range(1, J):
            acc_o = stt(P, f"ao{k}", xTk[k], alpha[:, J + k:J + k + 1], acc_o,
                        op0=MUL, op1=ADD, w=BL)
        y = stt(D, "y", acc_e, alpha[:, 0:1], acc_o, op0=ADD, op1=ADD, w=BL)

        SP.dma_start(out=out[:, :], in_=y)

        if debug_taps:
            for nm, t in [("d_gacc", gacc), ("d_gsum", g_sum), ("d_h", h),
                          ("d_alpha", alpha), ("d_acce", acc_e),
                          ("d_acco", acc_o)]:
                SP.dma_start(out=taps[nm][:, :], in_=t)

    nc.compile()
    return nc


_CACHE = {}


def _get_nc():
    if "nc" not in _CACHE:
        _CACHE["nc"] = build(with_collective=True)
    return _CACHE["nc"]


def kernel(x, cdf_data, bw_param):
    x = np.ascontiguousarray(x, dtype=np.float32)
    cdf_data = np.ascontiguousarray(cdf_data, dtype=np.float32)
    bw_param = np.ascontiguousarray(bw_param, dtype=np.float32)
    nc = _get_nc()

    xd = float(np.abs(x).max()) * 1.0005
    th = _cheb_theta()
    t_nodes = (xd * np.cos(th)).astype(np.float32)              # [N]
    bw = (1.0 / (1.0 + np.exp(-bw_param.astype(np.float64))))[0]
    a = (1.0 / (bw * math.sqrt(2.0))).astype(np.float32)        # [F]

    xt = np.clip(x.T, -xd, xd).astype(np.float32) / np.float32(xd)   # [F, B]
    wtf = (np.float32(2.0) * xt * xt - np.float32(1.0)).astype(np.float32)
    cdf_halves = [np.ascontiguousarray(cdf_data[h * SL:(h + 1) * SL].T)
                  for h in range(NSPL)]                          # each [F, SL]

    in_maps = []
    for i in range(N_CORES):
        g, h = i // NSPL, i % NSPL
        xw_i = np.concatenate([xt[:, i * BL:(i + 1) * BL],
                               wtf[:, i * BL:(i + 1) * BL]], axis=1)
        bias = a[:, None] * t_nodes[None, g * NLOC:(g + 1) * NLOC]  # [F, NLOC]
        consts_i = np.concatenate([-a[:, None], bias], axis=1)
        in_maps.append({
            "xw": np.ascontiguousarray(xw_i),
            "cdf_t": cdf_halves[h],
            "consts": np.ascontiguousarray(consts_i.astype(np.float32)),
        })
    res = bass_utils.run_bass_kernel_spmd(nc, in_maps, core_ids=list(range(N_CORES)))
    return np.concatenate([res.results[i]["out"].T for i in range(N_CORES)], axis=0)


# revision 14
# speedup vs baseline: 1.6813x; 1.2091x over previous
"""Trainium2 Bass kernel for nn_BatchPitNorm1d (pairwise Gaussian-CDF KDE + inverse-normal).

Math:  u[b,f] = mean_s Phi((x[b,f] - c[s,f]) / bw[f]),  out = ndtri(u),
       bw = sigmoid(bw_param).

Algorithm (v2): for fixed f, ndtri(u) is a smooth function H_f(x) of x alone,
so instead of B*S*F pairwise Phi evals the kernel:
  1. evaluates the erf-sums g_f(t) at N=12 Chebyshev nodes on a runtime-tight
     domain [-XD, XD] (XD = max|x|), sharded (4 node-groups) x (2 sample
     halves) over 8 cores - NLOC=3 nodes x 1024 samples per core, one fused
     ACT erf instruction per node (accum_out = free-dim sum, per-partition
     scale/bias precomputed on host),
  2. AllGathers the raw [F, NLOC] blocks (feature-major), reads them back
     with one 4D-AP DMA as [F, 2N] and adds the two sample-halves,
  3. applies ndtri at the nodes in feature-major [F, N] layout: central
     rational(3,3) on DVE, deg-5 log-domain tail polynomial on GpSimd (Pool),
     Ln on ACT (table load hidden under the gather), branchless blend via
     copy_predicated,
  4. transposes H via PE, fits per-feature even/odd Chebyshev coefficients
     with one PE matmul,
  5. evaluates y = sum_k ae_k T_k(w) + x~ * sum_k ao_k T_k(w), w = 2x~^2-1,
     with basis tiles T_k / x~T_k precomputed during the grid phase and two
     parallel per-partition-scalar accumulation chains (DVE even, Pool odd).

Host-side prep (cheap [F]-sized math): transpose/shard, x~ = x/XD, w, bw ->
erf scale/bias vectors.  Total error vs the f32 reference: rel ~9e-4
(gate 2e-2).
"""

import math
from contextlib import ExitStack

import numpy as np

import concourse.bass as bass
import concourse.bacc as bacc
import concourse.tile as tile
from concourse import mybir
from concourse import bass_utils

F32 = mybir.dt.float32
ADD = mybir.AluOpType.add
MUL = mybir.AluOpType.mult
SUB = mybir.AluOpType.subtract

N_CORES = 8
B, S, F = 512, 2048, 128
BL = B // N_CORES          # 64 batch rows per core
N_CHEB = 12                # Chebyshev nodes / polynomial order
NGRP = 4                   # node groups (cores 2g, 2g+1 share a node group)
NSPL = 2                   # sample splits (even core: half 0, odd: half 1)
NLOC = N_CHEB // NGRP      # 3 nodes per core
SL = S // NSPL             # 1024 samples per core
J = N_CHEB // 2            # even/odd coefficient count

GSCALE = 1.0 / (2.0 * S)
PLOW = 0.02425             # central/tail blend point (on v = min(u,1-u))
VCLAMP = 0.5 - 2.5e-6      # |q| clamp => v >= 2.5e-6 (empirical node min 5e-6)

# Central branch: ndtri(0.5+q) = q*N(r)/D(r), r = q^2, rational (3,3)
# fitted offline for v >= PLOW (max rel err ~1e-5).
CEN_NUM = [-14.41153095969586, 34.82754843726583, -17.684192118918105,
           2.5066372796948575]
CEN_DEN = [-14.220558591278943, 20.063397583232298, -8.101751140071201, 1.0]

# Tail branch: ndtri(v) = P(m), m = (ln v - TAIL_C)/TAIL_H, fitted on
# v in [1.5e-6, 0.0295] (deg 4, max abs err 1.5e-3; node-error sensitivity of
# the final interpolant is ~0.03 rel per unit, so this contributes ~5e-5).
TAIL_C = -8.466705232746236
TAIL_H = 4.943340217109873
TAIL_PW = [-3.52647668140479, 1.3038877066720616, 0.2096002924304478,
           0.08624246741498726, 0.037068735313101606]


def _cheb_theta():
    return (np.arange(N_CHEB) + 0.5) * np.pi / N_CHEB


def _fit_matrix():
    """Cfit[n, k] with alpha[f, k] = sum_n H[f, n] * Cfit[n, k].

    Basis columns 0..J-1 = even coeffs (T_j(w)), J..N-1 = odd (xt*T_j(w)),
    w = 2*xt^2-1, xt = normalized nodes.  XD-independent.
    """
    th = _cheb_theta()
    xt = np.cos(th)
    w = 2 * xt * xt - 1
    M = np.zeros((N_CHEB, N_CHEB))
    for j in range(J):
        M[:, j] = np.cos(j * np.arccos(np.clip(w, -1, 1)))
        M[:, J + j] = xt * M[:, j]
    return np.ascontiguousarray(np.linalg.inv(M).T).astype(np.float32)


def build(with_collective=True, debug_taps=False):
    nc = bacc.Bacc("TRN2", target_bir_lowering=False, debug=False,
                   enable_asserts=False, num_devices=N_CORES)

    # Inputs arrive pre-transposed (feature-major) from the host shard step.
    xw = nc.dram_tensor("xw", [F, 2 * BL], F32, kind="ExternalInput")       # xt | wt
    cdf_t = nc.dram_tensor("cdf_t", [F, SL], F32, kind="ExternalInput")
    consts = nc.dram_tensor("consts", [F, 1 + NLOC], F32, kind="ExternalInput")  # -a | a*t_j
    out = nc.dram_tensor("out", [F, BL], F32, kind="ExternalOutput")
    taps = {}
    if debug_taps:
        for nm, shp in [("d_gacc", [F, NLOC]), ("d_gsum", [F, N_CHEB]),
                        ("d_h", [F, N_CHEB]), ("d_alpha", [F, N_CHEB]),
                        ("d_acce", [F, BL]), ("d_acco", [F, BL])]:
            taps[nm] = nc.dram_tensor(nm, shp, F32, kind="ExternalOutput")

    cfit_h = nc.inline_tensor(_fit_matrix(), name="cfit")
    ident_h = nc.inline_tensor(np.eye(F, dtype=np.float32), name="ident")

    with tile.TileContext(nc) as tc, ExitStack() as ctx:
        sb = ctx.enter_context(tc.tile_pool(name="sb", bufs=1))
        psum = ctx.enter_context(tc.tile_pool(name="psum", bufs=1, space="PSUM"))
        dram = ctx.enter_context(tc.tile_pool(name="dram", bufs=1, space="DRAM"))

        D = nc.vector    # DVE
        P = nc.gpsimd    # Pool
        A = nc.scalar    # ACT
        SP = nc.sync     # SP

        def ts(eng, name, in0, s1, s2=None, op0=MUL, op1=ADD, w=N_CHEB):
            t = sb.tile([F, w], F32, name=name, tag=name)
            if s2 is None:
                eng.tensor_scalar(out=t, in0=in0, scalar1=s1, scalar2=None, op0=op0)
            else:
                eng.tensor_scalar(out=t, in0=in0, scalar1=s1, scalar2=s2,
                                  op0=op0, op1=op1)
            return t

        def stt(eng, name, in0, s, in1, op0=ADD, op1=MUL, w=N_CHEB, out=None):
            t = out if out is not None else sb.tile([F, w], F32, name=name, tag=name)
            eng.scalar_tensor_tensor(out=t, in0=in0, scalar=s, in1=in1,
                                     op0=op0, op1=op1)
            return t

        # ---------------- input DMAs (one per queue, issued up front)
        cT = sb.tile([F, SL], F32, name="cT")
        SP.dma_start(out=cT, in_=cdf_t[:, :])
        xw_sb = sb.tile([F, 2 * BL], F32, name="xw")
        P.dma_start(out=xw_sb, in_=xw[:, :])
        cst = sb.tile([F, 1 + NLOC], F32, name="cst")
        P.dma_start(out=cst, in_=consts[:, :])
        cfit_sb = sb.tile([N_CHEB, N_CHEB], F32, name="cfit")
        P.dma_start(out=cfit_sb, in_=cfit_h[:, :])
        ident_sb = sb.tile([F, F], F32, name="ident")
        P.dma_start(out=ident_sb, in_=ident_h[:, :])

        xt = xw_sb[:, :BL]
        wt = xw_sb[:, BL:]

        half_c = sb.tile([F, 1], F32, name="halfc")
        D.memset(half_c, 0.5)
        # Data-independent dummy erf: forces the erf table load at t~1us,
        # while the cdf DMA is still in flight (instead of right before erf0).
        erfdum = sb.tile([F, 1], F32, name="erfdum")
        A.activation(out=erfdum, in_=half_c,
                     func=mybir.ActivationFunctionType.Erf, scale=0.0,
                     bias=half_c[:, 0:1])

        # ---------------- basis precompute (hidden under grid phase)
        # Even basis T_k(w), odd basis xt*T_k(w); T0=1 and xT0=xt are implicit.
        wt2 = ts(D, "wt2", wt, 2.0, w=BL)
        wsq = stt(D, "wsq", wt, 0.0, wt, w=BL)
        T2 = ts(D, "T2", wsq, 2.0, -1.0, w=BL)
        Tk = {1: wt, 2: T2}
        for k in range(3, J):
            p = stt(D, f"Tp{k}", Tk[k - 1], 0.0, wt2, w=BL)
            Tk[k] = stt(D, f"T{k}", p, 0.0, Tk[k - 2], op1=SUB, w=BL)
        xTk = {}
        for k in range(1, J):
            xTk[k] = stt(P, f"xT{k}", Tk[k], 0.0, xt, w=BL)

        # ---------------- grid: gacc[f, j] = sum_s erf(-a_f*c_sf + a_f*t_j)
        gacc = sb.tile([F, NLOC], F32, name="gacc")
        scr = psum.tile([F, SL], F32, name="scr", tag="scr")
        for j in range(NLOC):
            A.activation(out=scr, in_=cT, func=mybir.ActivationFunctionType.Erf,
                         bias=cst[:, 1 + j:2 + j], scale=cst[:, 0:1],
                         accum_out=gacc[:, j:j + 1])
        # Force the Ln table switch right after the grid so the ~1.3us load
        # hides under the gather round-trip.  Reads the last accum column so
        # the scheduler cannot hoist it between the erfs (which would force
        # extra erf-table reloads).
        lndum = sb.tile([F, 1], F32, name="lndum")
        A.activation(out=lndum, in_=gacc[:, NLOC - 1:NLOC],
                     func=mybir.ActivationFunctionType.Ln, scale=0.0,
                     bias=half_c[:, 0:1])

        # ---------------- exchange: AllGather of the [F, NLOC] blocks
        cin = dram.tile([F, NLOC], F32, tag="cin")
        SP.dma_start(out=cin[:, :], in_=gacc)
        cout = dram.tile([N_CORES, F, NLOC], F32, tag="cout",
                         addr_space="Shared" if with_collective else "Local")
        if with_collective:
            P.collective_compute(
                "AllGather", mybir.AluOpType.bypass,
                replica_groups=[list(range(N_CORES))],
                ins=[cin.opt()], outs=[cout.opt()],
            )
        # Single readback of all 8 [F, NLOC] blocks, rank-major:
        # gbig[f, rank*NLOC + j] = cout[rank][f][j], rank = g*NSPL + h.
        gbig = sb.tile([F, N_CORES * NLOC], F32, name="gbig")
        if with_collective:
            src_ap = bass.AP(
                tensor=cout.tensor, offset=cout.offset,
                ap=[[NLOC, F], [F * NLOC, N_CORES], [1, NLOC]])
        else:  # stand-in: broadcast-read own block (timing model only)
            src_ap = bass.AP(
                tensor=cin.tensor, offset=cin.offset,
                ap=[[NLOC, F], [0, N_CORES], [1, NLOC]])
        A.dma_start(out=gbig[:, :], in_=src_ap)

        # g_sum[f, g*NLOC+j] = sum_h gbig[f, (g*NSPL+h)*NLOC + j]
        g_sum = sb.tile([F, N_CHEB], F32, name="gsum")
        gb_w = N_CORES * NLOC
        h0_ap = bass.AP(tensor=gbig.tensor, offset=gbig.offset,
                        ap=[[gb_w, F], [NSPL * NLOC, NGRP], [1, NLOC]])
        h1_ap = bass.AP(tensor=gbig.tensor, offset=gbig.offset + NLOC,
                        ap=[[gb_w, F], [NSPL * NLOC, NGRP], [1, NLOC]])
        D.scalar_tensor_tensor(out=g_sum, in0=h0_ap, scalar=0.0, in1=h1_ap,
                               op0=ADD, op1=ADD)

        # ---------------- ndtri at the nodes, feature-major [F, N]
        # DVE: q, r, |q|-clamp, central mask + rational(3,3)
        q = ts(D, "q", g_sum, GSCALE)
        r = stt(D, "r", q, 0.0, q)
        mn2 = ts(D, "mn2", q, 0.0, VCLAMP, op0=mybir.AluOpType.abs_max,
                 op1=mybir.AluOpType.min)
        mc = ts(D, "mc", mn2, 0.5 - PLOW, None, op0=mybir.AluOpType.is_le)
        # ACT: lnv = Ln(0.5 - |q|)
        lnv = sb.tile([F, N_CHEB], F32, name="lnv")
        A.activation(out=lnv, in_=mn2, func=mybir.ActivationFunctionType.Ln,
                     scale=-1.0, bias=half_c[:, 0:1])
        # central: q*N(r)/D(r)
        ca = ts(D, "ca0", r, float(CEN_NUM[0]))
        ca = stt(D, "ca1", ca, float(CEN_NUM[1]), r)
        ca = stt(D, "ca2", ca, float(CEN_NUM[2]), r)
        nq = stt(D, "nq", ca, float(CEN_NUM[3]), q)
        da = ts(D, "da0", r, float(CEN_DEN[0]))
        da = stt(D, "da1", da, float(CEN_DEN[1]), r)
        da = stt(D, "da2", da, float(CEN_DEN[2]), r)
        df = ts(D, "df", da, float(CEN_DEN[3]), None, op0=ADD)
        rec = sb.tile([F, N_CHEB], F32, name="rec")
        D.reciprocal(out=rec, in_=df)
        xc = stt(D, "xc", nq, 0.0, rec)
        # tail on Pool: P(m)*nsgn, m = (lnv - C)/H
        mge = ts(P, "mge", g_sum, 0.0, None, op0=mybir.AluOpType.is_ge)
        nsgn = ts(P, "nsgn", mge, -2.0, 1.0)
        m = ts(P, "m", lnv, 1.0 / TAIL_H, -TAIL_C / TAIL_H)
        hc = TAIL_PW[::-1]  # high -> low
        ta = ts(P, "ta0", m, float(hc[0]))
        for i, c in enumerate(hc[1:-1]):
            ta = stt(P, f"ta{i + 1}", ta, float(c), m)
        h = sb.tile([F, N_CHEB], F32, name="h")
        stt(P, "tsgn", ta, float(hc[-1]), nsgn, out=h)
        # blend: overwrite central region with xc
        D.copy_predicated(h, mc, xc)

        # ---------------- fit: alpha = h @ Cfit via PE transpose + matmul
        hT_ps = psum.tile([N_CHEB, F], F32, tag="hT")
        nc.tensor.transpose(hT_ps, h, ident_sb)
        hT_sb = sb.tile([N_CHEB, F], F32, name="hT")
        P.tensor_copy(out=hT_sb, in_=hT_ps)
        alpha_ps = psum.tile([F, N_CHEB], F32, tag="alpha")
        nc.tensor.matmul(out=alpha_ps, lhsT=hT_sb, rhs=cfit_sb,
                         start=True, stop=True)
        alpha = sb.tile([F, N_CHEB], F32, name="alpha")
        D.tensor_copy(out=alpha, in_=alpha_ps)

        # ---------------- evaluate: 2x2 split accumulation chains (even on
        # DVE, odd on Pool; two independent sub-chains per engine pipeline
        # without per-op write-ack stalls)
        e1 = ts(D, "ae1", wt, alpha[:, 1:2], None, w=BL)
        e1 = stt(D, "ae2", Tk[2], alpha[:, 2:3], e1, op0=MUL, op1=ADD, w=BL)
        e1 = stt(D, "ae3", Tk[3], alpha[:, 3:4], e1, op0=MUL, op1=ADD, w=BL)
        e2 = ts(D, "ae4", Tk[4], alpha[:, 4:5], None, w=BL)
        e2 = stt(D, "ae5", Tk[5], alpha[:, 5:6], e2, op0=MUL, op1=ADD, w=BL)
        o1 = ts(P, "ao0", xt, alpha[:, J:J + 1], None, w=BL)
        o1 = stt(P, "ao1", xTk[1], alpha[:, J + 1:J + 2], o1, op0=MUL, op1=ADD, w=BL)
        o1 = stt(P, "ao2", xTk[2], alpha[:, J + 2:J + 3], o1, op0=MUL, op1=ADD, w=BL)
        o2 = ts(P, "ao3", xTk[3], alpha[:, J + 3:J + 4], None, w=BL)
        o2 = stt(P, "ao4", xTk[4], alpha[:, J + 4:J + 5], o2, op0=MUL, op1=ADD, w=BL)
        o2 = stt(P, "ao5", xTk[5], alpha[:, J + 5:J + 6], o2, op0=MUL, op1=ADD, w=BL)
        acc_o = stt(P, "aco", o1, 0.0, o2, op0=ADD, op1=ADD, w=BL)
        ye = stt(D, "ye", e1, alpha[:, 0:1], e2, op0=ADD, op1=ADD, w=BL)
        y = stt(D, "y", ye, 0.0, acc_o, op0=ADD, op1=ADD, w=BL)

        SP.dma_start(out=out[:, :], in_=y)

        if debug_taps:
            for nm, t in [("d_gacc", gacc), ("d_gsum", g_sum), ("d_h", h),
                          ("d_alpha", alpha), ("d_acce", ye),
                          ("d_acco", acc_o)]:
                SP.dma_start(out=taps[nm][:, :], in_=t)

    nc.compile()
    return nc


_CACHE = {}


def _get_nc():
    if "nc" not in _CACHE:
        _CACHE["nc"] = build(with_collective=True)
    return _CACHE["nc"]


def kernel(x, cdf_data, bw_param):
    x = np.ascontiguousarray(x, dtype=np.float32)
    cdf_data = np.ascontiguousarray(cdf_data, dtype=np.float32)
    bw_param = np.ascontiguousarray(bw_param, dtype=np.float32)
    nc = _get_nc()

    xd = float(np.abs(x).max()) * 1.0005
    th = _cheb_theta()
    t_nodes = (xd * np.cos(th)).astype(np.float32)              # [N]
    bw = (1.0 / (1.0 + np.exp(-bw_param.astype(np.float64))))[0]
    a = (1.0 / (bw * math.sqrt(2.0))).astype(np.float32)        # [F]

    xt = np.clip(x.T, -xd, xd).astype(np.float32) / np.float32(xd)   # [F, B]
    wtf = (np.float32(2.0) * xt * xt - np.float32(1.0)).astype(np.float32)
    cdf_halves = [np.ascontiguousarray(cdf_data[h * SL:(h + 1) * SL].T)
                  for h in range(NSPL)]                          # each [F, SL]

    in_maps = []
    for i in range(N_CORES):
        g, h = i // NSPL, i % NSPL
        xw_i = np.concatenate([xt[:, i * BL:(i + 1) * BL],
                               wtf[:, i * BL:(i + 1) * BL]], axis=1)
        bias = a[:, None] * t_nodes[None, g * NLOC:(g + 1) * NLOC]  # [F, NLOC]
        consts_i = np.concatenate([-a[:, None], bias], axis=1)
        in_maps.append({
            "xw": np.ascontiguousarray(xw_i),
            "cdf_t": cdf_halves[h],
            "consts": np.ascontiguousarray(consts_i.astype(np.float32)),
        })
    res = bass_utils.run_bass_kernel_spmd(nc, in_maps, core_ids=list(range(N_CORES)))
    return np.concatenate([res.results[i]["out"].T for i in range(N_CORES)], axis=0)


# revision 16
# speedup vs baseline: 1.7070x; 1.0153x over previous
"""Trainium2 Bass kernel for nn_BatchPitNorm1d (pairwise Gaussian-CDF KDE + inverse-normal).

Math:  u[b,f] = mean_s Phi((x[b,f] - c[s,f]) / bw[f]),  out = ndtri(u),
       bw = sigmoid(bw_param).

Algorithm (v2): for fixed f, ndtri(u) is a smooth function H_f(x) of x alone,
so instead of B*S*F pairwise Phi evals the kernel:
  1. evaluates the erf-sums g_f(t) at N=12 Chebyshev nodes on a runtime-tight
     domain [-XD, XD] (XD = max|x|), sharded (4 node-groups) x (2 sample
     halves) over 8 cores - NLOC=3 nodes x 1024 samples per core, one fused
     ACT erf instruction per node (accum_out = free-dim sum, per-partition
     scale/bias precomputed on host),
  2. AllGathers the raw [F, NLOC] blocks (feature-major), reads them back
     with one 4D-AP DMA as [F, 2N] and adds the two sample-halves,
  3. applies ndtri at the nodes in feature-major [F, N] layout: central
     rational(3,3) on DVE, deg-5 log-domain tail polynomial on GpSimd (Pool),
     Ln on ACT (table load hidden under the gather), branchless blend via
     copy_predicated,
  4. transposes H via PE, fits per-feature even/odd Chebyshev coefficients
     with one PE matmul,
  5. evaluates y = sum_k ae_k T_k(w) + x~ * sum_k ao_k T_k(w), w = 2x~^2-1,
     with basis tiles T_k / x~T_k precomputed during the grid phase and two
     parallel per-partition-scalar accumulation chains (DVE even, Pool odd).

Host-side prep (cheap [F]-sized math): transpose/shard, x~ = x/XD, w, bw ->
erf scale/bias vectors.  Total error vs the f32 reference: rel ~9e-4
(gate 2e-2).
"""

import math
from contextlib import ExitStack

import numpy as np

import concourse.bass as bass
import concourse.bacc as bacc
import concourse.tile as tile
from concourse import mybir
from concourse import bass_utils

F32 = mybir.dt.float32
ADD = mybir.AluOpType.add
MUL = mybir.AluOpType.mult
SUB = mybir.AluOpType.subtract

N_CORES = 8
B, S, F = 512, 2048, 128
BL = B // N_CORES          # 64 batch rows per core
N_CHEB = 12                # Chebyshev nodes / polynomial order
NGRP = 4                   # node groups (cores 2g, 2g+1 share a node group)
NSPL = 2                   # sample splits (even core: half 0, odd: half 1)
NLOC = N_CHEB // NGRP      # 3 nodes per core
SL = S // NSPL             # 1024 samples per core
J = N_CHEB // 2            # even/odd coefficient count

GSCALE = 1.0 / (2.0 * S)
PLOW = 0.02425             # central/tail blend point (on v = min(u,1-u))
VCLAMP = 0.5 - 2.5e-6      # |q| clamp => v >= 2.5e-6 (empirical node min 5e-6)

# Central branch: ndtri(0.5+q) = q*N(r)/D(r), r = q^2, rational (3,3)
# fitted offline for v >= PLOW (max rel err ~1e-5).
CEN_NUM = [-14.41153095969586, 34.82754843726583, -17.684192118918105,
           2.5066372796948575]
CEN_DEN = [-14.220558591278943, 20.063397583232298, -8.101751140071201, 1.0]

# Tail branch: ndtri(v) = P(m), m = (ln v - TAIL_C)/TAIL_H, fitted on
# v in [1.5e-6, 0.0295] (deg 4, max abs err 1.5e-3; node-error sensitivity of
# the final interpolant is ~0.03 rel per unit, so this contributes ~5e-5).
TAIL_C = -8.466705232746236
TAIL_H = 4.943340217109873
TAIL_PW = [-3.52647668140479, 1.3038877066720616, 0.2096002924304478,
           0.08624246741498726, 0.037068735313101606]


def _cheb_theta():
    return (np.arange(N_CHEB) + 0.5) * np.pi / N_CHEB


def _fit_matrix():
    """Cfit[n, k] with alpha[f, k] = sum_n H[f, n] * Cfit[n, k].

    Basis columns 0..J-1 = even coeffs (T_j(w)), J..N-1 = odd (xt*T_j(w)),
    w = 2*xt^2-1, xt = normalized nodes.  XD-independent.
    """
    th = _cheb_theta()
    xt = np.cos(th)
    w = 2 * xt * xt - 1
    M = np.zeros((N_CHEB, N_CHEB))
    for j in range(J):
        M[:, j] = np.cos(j * np.arccos(np.clip(w, -1, 1)))
        M[:, J + j] = xt * M[:, j]
    return np.ascontiguousarray(np.linalg.inv(M).T).astype(np.float32)


def build(with_collective=True, debug_taps=False):
    nc = bacc.Bacc("TRN2", target_bir_lowering=False, debug=False,
                   enable_asserts=False, num_devices=N_CORES)

    # Inputs arrive pre-transposed (feature-major) from the host shard step.
    xw = nc.dram_tensor("xw", [F, 2 * BL], F32, kind="ExternalInput")       # xt | wt
    cdf_t = nc.dram_tensor("cdf_t", [F, SL], F32, kind="ExternalInput")
    consts = nc.dram_tensor("consts", [F, 1 + NLOC], F32, kind="ExternalInput")  # -a | a*t_j
    out = nc.dram_tensor("out", [F, BL], F32, kind="ExternalOutput")
    taps = {}
    if debug_taps:
        for nm, shp in [("d_gacc", [F, NLOC]), ("d_gsum", [F, N_CHEB]),
                        ("d_h", [F, N_CHEB]), ("d_alpha", [F, N_CHEB]),
                        ("d_acce", [F, BL]), ("d_acco", [F, BL])]:
            taps[nm] = nc.dram_tensor(nm, shp, F32, kind="ExternalOutput")

    cfit_h = nc.inline_tensor(_fit_matrix(), name="cfit")
    ident_h = nc.inline_tensor(np.eye(F, dtype=np.float32), name="ident")

    with tile.TileContext(nc) as tc, ExitStack() as ctx:
        sb = ctx.enter_context(tc.tile_pool(name="sb", bufs=1))
        psum = ctx.enter_context(tc.tile_pool(name="psum", bufs=1, space="PSUM"))
        dram = ctx.enter_context(tc.tile_pool(name="dram", bufs=1, space="DRAM"))

        D = nc.vector    # DVE
        P = nc.gpsimd    # Pool
        A = nc.scalar    # ACT
        SP = nc.sync     # SP

        def ts(eng, name, in0, s1, s2=None, op0=MUL, op1=ADD, w=N_CHEB):
            t = sb.tile([F, w], F32, name=name, tag=name)
            if s2 is None:
                eng.tensor_scalar(out=t, in0=in0, scalar1=s1, scalar2=None, op0=op0)
            else:
                eng.tensor_scalar(out=t, in0=in0, scalar1=s1, scalar2=s2,
                                  op0=op0, op1=op1)
            return t

        def stt(eng, name, in0, s, in1, op0=ADD, op1=MUL, w=N_CHEB, out=None):
            t = out if out is not None else sb.tile([F, w], F32, name=name, tag=name)
            eng.scalar_tensor_tensor(out=t, in0=in0, scalar=s, in1=in1,
                                     op0=op0, op1=op1)
            return t

        # ---------------- input DMAs (one per queue, issued up front)
        cT = sb.tile([F, SL], F32, name="cT")
        SP.dma_start(out=cT, in_=cdf_t[:, :])
        cst = sb.tile([F, 1 + NLOC], F32, name="cst")
        P.dma_start(out=cst, in_=consts[:, :])
        xw_sb = sb.tile([F, 2 * BL], F32, name="xw")
        P.dma_start(out=xw_sb, in_=xw[:, :])
        cfit_sb = sb.tile([N_CHEB, N_CHEB], F32, name="cfit")
        P.dma_start(out=cfit_sb, in_=cfit_h[:, :])
        ident_sb = sb.tile([F, F], F32, name="ident")
        P.dma_start(out=ident_sb, in_=ident_h[:, :])

        xt = xw_sb[:, :BL]
        wt = xw_sb[:, BL:]

        half_c = sb.tile([F, 1], F32, name="halfc")
        D.memset(half_c, 0.5)
        # Data-independent dummy erf: forces the erf table load at t~1us,
        # while the cdf DMA is still in flight (instead of right before erf0).
        erfdum = sb.tile([F, 1], F32, name="erfdum")
        A.activation(out=erfdum, in_=half_c,
                     func=mybir.ActivationFunctionType.Erf, scale=0.0,
                     bias=half_c[:, 0:1])

        # ---------------- basis precompute (hidden under grid phase)
        # Even basis T_k(w), odd basis xt*T_k(w); T0=1 and xT0=xt are implicit.
        wt2 = ts(D, "wt2", wt, 2.0, w=BL)
        wsq = stt(D, "wsq", wt, 0.0, wt, w=BL)
        T2 = ts(D, "T2", wsq, 2.0, -1.0, w=BL)
        Tk = {1: wt, 2: T2}
        for k in range(3, J):
            p = stt(D, f"Tp{k}", Tk[k - 1], 0.0, wt2, w=BL)
            Tk[k] = stt(D, f"T{k}", p, 0.0, Tk[k - 2], op1=SUB, w=BL)
        xTk = {}
        for k in range(1, J):
            xTk[k] = stt(P, f"xT{k}", Tk[k], 0.0, xt, w=BL)

        # ---------------- grid: gacc[f, j] = sum_s erf(-a_f*c_sf + a_f*t_j)
        gacc = sb.tile([F, NLOC], F32, name="gacc")
        scr = psum.tile([F, SL], F32, name="scr", tag="scr")
        for j in range(NLOC):
            A.activation(out=scr, in_=cT, func=mybir.ActivationFunctionType.Erf,
                         bias=cst[:, 1 + j:2 + j], scale=cst[:, 0:1],
                         accum_out=gacc[:, j:j + 1])
        # Force the Ln table switch right after the grid so the ~1.3us load
        # hides under the gather round-trip.  Reads the last accum column so
        # the scheduler cannot hoist it between the erfs (which would force
        # extra erf-table reloads).
        lndum = sb.tile([F, 1], F32, name="lndum")
        A.activation(out=lndum, in_=gacc[:, NLOC - 1:NLOC],
                     func=mybir.ActivationFunctionType.Ln, scale=0.0,
                     bias=half_c[:, 0:1])

        # ---------------- exchange: AllGather of the [F, NLOC] blocks
        cin = dram.tile([F, NLOC], F32, tag="cin")
        SP.dma_start(out=cin[:, :], in_=gacc)
        cout = dram.tile([N_CORES, F, NLOC], F32, tag="cout",
                         addr_space="Shared" if with_collective else "Local")
        if with_collective:
            P.collective_compute(
                "AllGather", mybir.AluOpType.bypass,
                replica_groups=[list(range(N_CORES))],
                ins=[cin.opt()], outs=[cout.opt()],
            )
        # Single readback of all 8 [F, NLOC] blocks, rank-major:
        # gbig[f, rank*NLOC + j] = cout[rank][f][j], rank = g*NSPL + h.
        gbig = sb.tile([F, N_CORES * NLOC], F32, name="gbig")
        if with_collective:
            src_ap = bass.AP(
                tensor=cout.tensor, offset=cout.offset,
                ap=[[NLOC, F], [F * NLOC, N_CORES], [1, NLOC]])
        else:  # stand-in: broadcast-read own block (timing model only)
            src_ap = bass.AP(
                tensor=cin.tensor, offset=cin.offset,
                ap=[[NLOC, F], [0, N_CORES], [1, NLOC]])
        A.dma_start(out=gbig[:, :], in_=src_ap)

        # g_sum[f, g*NLOC+j] = sum_h gbig[f, (g*NSPL+h)*NLOC + j]
        g_sum = sb.tile([F, N_CHEB], F32, name="gsum")
        gb_w = N_CORES * NLOC
        h0_ap = bass.AP(tensor=gbig.tensor, offset=gbig.offset,
                        ap=[[gb_w, F], [NSPL * NLOC, NGRP], [1, NLOC]])
        h1_ap = bass.AP(tensor=gbig.tensor, offset=gbig.offset + NLOC,
                        ap=[[gb_w, F], [NSPL * NLOC, NGRP], [1, NLOC]])
        D.scalar_tensor_tensor(out=g_sum, in0=h0_ap, scalar=0.0, in1=h1_ap,
                               op0=ADD, op1=ADD)

        # ---------------- ndtri at the nodes, feature-major [F, N]
        # gscale = 1/(2S) = 2^-12 is an exact power of two, so it is folded
        # into the rational coefficients (exact f32 scaling): work directly on
        # r' = g^2 and finish with *g instead of computing q = g*gscale.
        CN = [CEN_NUM[i] * GSCALE ** (2 * (3 - i) + 1) for i in range(4)]
        CD = [CEN_DEN[i] * GSCALE ** (2 * (3 - i)) for i in range(4)]
        r2 = stt(D, "r2", g_sum, 0.0, g_sum)
        mn2 = ts(D, "mn2", g_sum, 0.0, GSCALE, op0=mybir.AluOpType.abs_max,
                 op1=MUL)  # |q| = |g|*gscale; v = 0.5-|q| >= ~5e-6 empirically
        mc = ts(D, "mc", mn2, 0.5 - PLOW, None, op0=mybir.AluOpType.is_le)
        # ACT: lnv = Ln(0.5 - |q|)
        lnv = sb.tile([F, N_CHEB], F32, name="lnv")
        A.activation(out=lnv, in_=mn2, func=mybir.ActivationFunctionType.Ln,
                     scale=-1.0, bias=half_c[:, 0:1])
        # central: q*N(r)/D(r) in the scaled variables
        ca = ts(D, "ca0", r2, float(CN[0]))
        ca = stt(D, "ca1", ca, float(CN[1]), r2)
        ca = stt(D, "ca2", ca, float(CN[2]), r2)
        nq = stt(D, "nq", ca, float(CN[3]), g_sum)
        da = ts(D, "da0", r2, float(CD[0]))
        da = stt(D, "da1", da, float(CD[1]), r2)
        da = stt(D, "da2", da, float(CD[2]), r2)
        df = ts(D, "df", da, float(CD[3]), None, op0=ADD)
        rec = sb.tile([F, N_CHEB], F32, name="rec")
        D.reciprocal(out=rec, in_=df)
        xc = stt(D, "xc", nq, 0.0, rec)
        # tail on Pool: P(m)*nsgn, m = (lnv - C)/H
        mge = ts(P, "mge", g_sum, 0.0, None, op0=mybir.AluOpType.is_ge)
        nsgn = ts(P, "nsgn", mge, -2.0, 1.0)
        m = ts(P, "m", lnv, 1.0 / TAIL_H, -TAIL_C / TAIL_H)
        hc = TAIL_PW[::-1]  # high -> low
        ta = ts(P, "ta0", m, float(hc[0]))
        for i, c in enumerate(hc[1:-1]):
            ta = stt(P, f"ta{i + 1}", ta, float(c), m)
        h = sb.tile([F, N_CHEB], F32, name="h")
        stt(P, "tsgn", ta, float(hc[-1]), nsgn, out=h)
        # blend: overwrite central region with xc
        D.copy_predicated(h, mc, xc)

        # ---------------- fit: alpha = h @ Cfit via PE transpose + matmul
        hT_ps = psum.tile([N_CHEB, F], F32, tag="hT")
        nc.tensor.transpose(hT_ps, h, ident_sb)
        hT_sb = sb.tile([N_CHEB, F], F32, name="hT")
        P.tensor_copy(out=hT_sb, in_=hT_ps)
        alpha_ps = psum.tile([F, N_CHEB], F32, tag="alpha")
        nc.tensor.matmul(out=alpha_ps, lhsT=hT_sb, rhs=cfit_sb,
                         start=True, stop=True)
        alpha = sb.tile([F, N_CHEB], F32, name="alpha")
        D.tensor_copy(out=alpha, in_=alpha_ps)

        # ---------------- evaluate: 2x2 split accumulation chains (even on
        # DVE, odd on Pool; two independent sub-chains per engine pipeline
        # without per-op write-ack stalls)
        e1 = ts(D, "ae1", wt, alpha[:, 1:2], None, w=BL)
        e1 = stt(D, "ae2", Tk[2], alpha[:, 2:3], e1, op0=MUL, op1=ADD, w=BL)
        e1 = stt(D, "ae3", Tk[3], alpha[:, 3:4], e1, op0=MUL, op1=ADD, w=BL)
        e2 = ts(D, "ae4", Tk[4], alpha[:, 4:5], None, w=BL)
        e2 = stt(D, "ae5", Tk[5], alpha[:, 5:6], e2, op0=MUL, op1=ADD, w=BL)
        o1 = ts(P, "ao0", xt, alpha[:, J:J + 1], None, w=BL)
        o1 = stt(P, "ao1", xTk[1], alpha[:, J + 1:J + 2], o1, op0=MUL, op1=ADD, w=BL)
        o1 = stt(P, "ao2", xTk[2], alpha[:, J + 2:J + 3], o1, op0=MUL, op1=ADD, w=BL)
        o2 = ts(P, "ao3", xTk[3], alpha[:, J + 3:J + 4], None, w=BL)
        o2 = stt(P, "ao4", xTk[4], alpha[:, J + 4:J + 5], o2, op0=MUL, op1=ADD, w=BL)
        o2 = stt(P, "ao5", xTk[5], alpha[:, J + 5:J + 6], o2, op0=MUL, op1=ADD, w=BL)
        acc_o = stt(P, "aco", o1, 0.0, o2, op0=ADD, op1=ADD, w=BL)
        ye = stt(D, "ye", e1, alpha[:, 0:1], e2, op0=ADD, op1=ADD, w=BL)
        y = stt(D, "y", ye, 0.0, acc_o, op0=ADD, op1=ADD, w=BL)

        SP.dma_start(out=out[:, :], in_=y)

        if debug_taps:
            for nm, t in [("d_gacc", gacc), ("d_gsum", g_sum), ("d_h", h),
                          ("d_alpha", alpha), ("d_acce", ye),
                          ("d_acco", acc_o)]:
                SP.dma_start(out=taps[nm][:, :], in_=t)

    nc.compile()
    return nc


_CACHE = {}


def _get_nc():
    if "nc" not in _CACHE:
        _CACHE["nc"] = build(with_collective=True)
    return _CACHE["nc"]


def kernel(x, cdf_data, bw_param):
    x = np.ascontiguousarray(x, dtype=np.float32)
    cdf_data = np.ascontiguousarray(cdf_data, dtype=np.float32)
    bw_param = np.ascontiguousarray(bw_param, dtype=np.float32)
    nc = _get_nc()

    xd = float(np.abs(x).max()) * 1.0005
    th = _cheb_theta()
    t_nodes = (xd * np.cos(th)).astype(np.float32)              # [N]
    bw = (1.0 / (1.0 + np.exp(-bw_param.astype(np.float64))))[0]
    a = (1.0 / (bw * math.sqrt(2.0))).astype(np.float32)        # [F]

    xt = np.clip(x.T, -xd, xd).astype(np.float32) / np.float32(xd)   # [F, B]
    wtf = (np.float32(2.0) * xt * xt - np.float32(1.0)).astype(np.float32)
    cdf_halves = [np.ascontiguousarray(cdf_data[h * SL:(h + 1) * SL].T)
                  for h in range(NSPL)]                          # each [F, SL]

    in_maps = []
    for i in range(N_CORES):
        g, h = i // NSPL, i % NSPL
        xw_i = np.concatenate([xt[:, i * BL:(i + 1) * BL],
                               wtf[:, i * BL:(i + 1) * BL]], axis=1)
        bias = a[:, None] * t_nodes[None, g * NLOC:(g + 1) * NLOC]  # [F, NLOC]
        consts_i = np.concatenate([-a[:, None], bias], axis=1)
        in_maps.append({
            "xw": np.ascontiguousarray(xw_i),
            "cdf_t": cdf_halves[h],
            "consts": np.ascontiguousarray(consts_i.astype(np.float32)),
        })
    res = bass_utils.run_bass_kernel_spmd(nc, in_maps, core_ids=list(range(N_CORES)))
    return np.concatenate([res.results[i]["out"].T for i in range(N_CORES)], axis=0)
